# revision 35
# baseline (speedup 1.0000x reference)
"""Bass/Trainium2 kernel for PharmRecGVP (GVP message-passing GNN), 8-core SPMD.

Sharding: edge-cut graph partition per the hint. Core c owns pharm[512c:512(c+1)]
and prot[2048c:2048(c+1)], computes messages for edges whose dst it owns
(dst-sorted, padded into fixed per-dst-tile edge budgets so the program
structure is identical across cores), updates its nodes, and exchanges updated
node features with one 8-core AllGather per conv layer (layers 0..2 only).

Compute layout: feature-major ([feat<=128 partitions, edges free]) so every GVP
linear is a TensorEngine matmul with stationary weights. Vector-channel einsums
use host-expanded block-diagonal weights (c-outer: vector row = c*V + u).
Gathers are indirect row DMAs; scatter (segment-sum) is indicator matmuls
accumulated in PSUM, indicators built on device via is_equal against an iota
constant. Edge geometry (unit vectors / distances) is static -> host-computed.
"""

import sys

for p in ("/opt/trn_rl_repo", "/opt/trn_rl_repo/concourse"):
    if p not in sys.path:
        sys.path.insert(0, p)

import numpy as np

N_CORES = 8
S, V = 128, 16
VF = 3 * V          # 48 vector feature cols (c-outer)
F = S + VF          # 176 packed feature cols
N_PH, N_PR = 4096, 16384
NLOC = {"ph": N_PH // N_CORES, "pr": N_PR // N_CORES}       # 512, 2048
NDT = {"ph": NLOC["ph"] // 128, "pr": NLOC["pr"] // 128}    # 4, 16
NLT = NLOC["ph"] + NLOC["pr"]                               # 2560
EDGES = {"ff": ("ph", "ph"), "pf": ("pr", "ph"),
         "fp": ("ph", "pr"), "pp": ("pr", "pr")}
N_CONVS, N_NOISE = 4, 3
MSG_NORM = 10.0
INTER_S, OUT_S = 64, 128
SEG = 64            # edge-budget granularity per dst tile
CH = 512            # edge chunk (matmul free dim)
F32 = np.float32


# ------------------------------------------------------------------ host prep
def _pack_tables(ph_s, ph_v, pr_s, pr_v):
    def pack(s, v):
        n = s.shape[0]
        vc = np.transpose(np.asarray(v, F32), (0, 2, 1)).reshape(n, VF)
        return np.concatenate([np.asarray(s, F32), vc], axis=1)
    return pack(ph_s, ph_v), pack(pr_s, pr_v)


def _expand_vec_weight(W, vin, vout, in_map=None):
    if in_map is None:
        in_map = [(c, v) for c in range(3) for v in range(vin)]
    We = np.zeros((len(in_map), 3 * vout), F32)
    for r, (c, v) in enumerate(in_map):
        We[r, c * vout:(c + 1) * vout] = W[v]
    return We


class WBlob:
    def __init__(self):
        self.cols, self.off = [], 0

    def add(self, w):
        w = np.asarray(w, F32)
        K, M = w.shape
        assert K <= 128, (K, M)
        buf = np.zeros((128, M), F32)
        buf[:K] = w
        off = self.off
        self.cols.append(buf)
        self.off += M
        return (off, K, M)

    def finish(self):
        return np.ascontiguousarray(np.concatenate(self.cols, axis=1))


def _prep_gvp(blob, p, vin, sin, in_map=None, vsplits=None):
    Wh, Wu = np.asarray(p["Wh"], F32), np.asarray(p["Wu"], F32)
    W, b = np.asarray(p["W"], F32), np.asarray(p["b"], F32)
    Wg, bg = np.asarray(p["Wg"], F32), np.asarray(p["bg"], F32)
    h, vout, sout = Wh.shape[1], Wu.shape[1], W.shape[1]
    g = dict(h=h, vout=vout, sout=sout, vin=vin, sin=sin)
    Whe = _expand_vec_weight(Wh, vin, h, in_map)
    if vsplits is None:
        vsplits = [Whe.shape[0]]
    whe_specs, r = [], 0
    for k in vsplits:
        whe_specs.append(blob.add(Whe[r:r + k]))
        r += k
    assert r == Whe.shape[0]
    g["Whe"] = whe_specs
    g["Wue"] = blob.add(_expand_vec_weight(Wu, h, vout))
    splits, r = [], 0
    while r < sin:
        k = min(128, sin - r)
        splits.append((blob.add(W[r:r + k]), k))
        r += k
    g["Ws"] = splits
    g["Wx"] = blob.add(W[sin:sin + h])
    g["Wg"] = blob.add(np.tile(Wg, (1, 3)))        # gate tri-replicated (c-outer)
    g["b"] = blob.add(b[:, None])
    g["bg"] = blob.add(np.tile(bg, 3)[:, None])
    return g


def _prep_params(params):
    blob = WBlob()
    # msg-GVP1 input rows: v0T (c-outer, v=u+1) first, then 3 unit rows (v=0)
    fmap = [(c, u + 1) for c in range(3) for u in range(V)] + \
           [(c, 0) for c in range(3)]
    umap = [(c, u) for c in range(3) for u in range(V)] + \
           [(c, V + u) for c in range(3) for u in range(V)]
    net = {"convs": []}
    for lp in params["convs"]:
        msg, upd = {}, {}
        for et in EDGES:
            gs = []
            for i, gp in enumerate(lp["msg"][et]):
                gs.append(_prep_gvp(blob, gp, V + 1 if i == 0 else V,
                                    S + 1 if i == 0 else S,
                                    fmap if i == 0 else None))
            msg[et] = gs
        for nt, key in (("ph", "pharm"), ("pr", "prot")):
            gs = []
            for i, gp in enumerate(lp["upd"][key]):
                gs.append(_prep_gvp(
                    blob, gp, 2 * V if i == 0 else V,
                    2 * S if i == 0 else S,
                    umap if i == 0 else None,
                    [VF, VF] if i == 0 else None))
            upd[nt] = gs
        net["convs"].append({"msg": msg, "upd": upd})
    net["noise"] = [_prep_gvp(blob, gp, V, S, None) for gp in params["noise"]]
    net["W_out"] = blob.add(np.asarray(params["W_out"], F32))
    net["b_out_rep"] = blob.add(
        np.tile(np.asarray(params["b_out"], F32)[None, :], (128, 1)))
    net["iota"] = blob.add(np.tile(np.arange(128, dtype=F32)[None, :], (128, 1)))
    net["ident"] = blob.add(np.eye(128, dtype=F32))
    net["sumh"] = {}
    for h in (16, 17, 32):
        Sm = np.zeros((3 * h, h), F32)
        for c in range(3):
            Sm[c * h:(c + 1) * h] = np.eye(h, dtype=F32)
        net["sumh"][h] = blob.add(Sm)
    return net, blob.finish()


def _prep_edges(inputs):
    xs = {"ph": np.asarray(inputs["pharm_x"], F32),
          "pr": np.asarray(inputs["prot_x"], F32)}
    percore = [dict() for _ in range(N_CORES)]
    meta = {}
    for et, (st, dt) in EDGES.items():
        src = np.asarray(inputs[f"{et}_src"]).astype(np.int64)
        dst = np.asarray(inputs[f"{et}_dst"]).astype(np.int64)
        nloc_d, ndt = NLOC[dt], NDT[dt]
        segs = np.zeros((N_CORES, ndt), np.int64)
        core_edges = []
        for c in range(N_CORES):
            m = (dst // nloc_d) == c
            s_c, d_c = src[m], dst[m] - c * nloc_d
            order = np.argsort(d_c, kind="stable")
            s_c, d_c = s_c[order], d_c[order]
            core_edges.append((s_c, d_c))
            segs[c] = (np.bincount(d_c // 128, minlength=ndt) + SEG - 1) // SEG
        budget = np.maximum(segs.max(axis=0), 1)
        starts = np.concatenate([[0], np.cumsum(budget * SEG)])
        e_pad = ((int(starts[-1]) + CH - 1) // CH) * CH
        ntiles = e_pad // 128
        triples = []  # (block, dtile)
        for dtile in range(ndt):
            lo, hi = int(starts[dtile]), int(starts[dtile + 1])
            for blk in range(lo // 128, (hi + 127) // 128):
                triples.append((blk, dtile))
        meta[et] = dict(e_pad=e_pad, ntiles=ntiles, nchunks=e_pad // CH,
                        triples=triples, ndt=ndt, starts=starts)
        ntri = len(triples)
        for c in range(N_CORES):
            s_c, d_c = core_edges[c]
            src_pad = np.zeros(e_pad, np.int64)
            dst_pad = np.full(e_pad, -1, np.int64)
            cnt = np.bincount(d_c // 128, minlength=ndt)
            pos = 0
            for dtile in range(ndt):
                k, lo = int(cnt[dtile]), int(starts[dtile])
                src_pad[lo:lo + k] = s_c[pos:pos + k]
                dst_pad[lo:lo + k] = d_c[pos:pos + k]
                pos += k
            xs_s = xs[st][src_pad]
            xs_d = xs[dt][np.where(dst_pad >= 0, dst_pad + c * nloc_d, 0)]
            xdiff = np.where((dst_pad >= 0)[:, None], xs_d - xs_s, 0.0)
            dd = np.linalg.norm(xdiff, axis=-1, keepdims=True)
            unit = xdiff / (dd + 1e-8)
            unitd = np.concatenate([dd, unit], axis=1)  # row0 = d (base-0 rhs)
            idx0 = src_pad.astype(np.int32)
            oc = src_pad // NLOC[st]
            base = 0 if st == "ph" else NLOC["ph"]
            idx1 = (oc * NLT + base + src_pad % NLOC[st]).astype(np.int32)
            rel = np.full((ntri, 128), -1.0, F32)
            for k, (blk, dtile) in enumerate(triples):
                seg = dst_pad[blk * 128:(blk + 1) * 128]
                loc = seg - dtile * 128
                ok = (seg >= 0) & (loc >= 0) & (loc < 128)
                rel[k] = np.where(ok, loc, -1).astype(F32)
            percore[c][f"{et}_idx"] = np.ascontiguousarray(np.stack(
                [idx0.reshape(ntiles, 128).T, idx1.reshape(ntiles, 128).T]))
            percore[c][f"{et}_unitd"] = np.ascontiguousarray(unitd.T.astype(F32))
            percore[c][f"{et}_rel"] = np.ascontiguousarray(rel.T)
    return percore, meta


# ------------------------------------------------------------- device program
def build_program(net, wblob_np, meta):
    import concourse.bass as bass
    import concourse.mybir as mybir
    from concourse import bacc
    from concourse.tile import TileContext

    dt = mybir.dt
    AF = mybir.ActivationFunctionType
    OP = mybir.AluOpType

    nc = bacc.Bacc(None, target_bir_lowering=False, num_devices=N_CORES)

    tab = {"ph": nc.dram_tensor("tab_ph", [N_PH, F], dt.float32, kind="ExternalInput"),
           "pr": nc.dram_tensor("tab_pr", [N_PR, F], dt.float32, kind="ExternalInput")}
    own = {"ph": nc.dram_tensor("own_ph", [NLOC["ph"], F], dt.float32, kind="ExternalInput"),
           "pr": nc.dram_tensor("own_pr", [NLOC["pr"], F], dt.float32, kind="ExternalInput")}
    ein = {}
    for et in EDGES:
        m = meta[et]
        ein[et] = dict(
            idx=nc.dram_tensor(f"{et}_idx", [2, 128, m["ntiles"]], dt.int32,
                               kind="ExternalInput"),
            unitd=nc.dram_tensor(f"{et}_unitd", [4, m["e_pad"]], dt.float32,
                                 kind="ExternalInput"),
            rel=nc.dram_tensor(f"{et}_rel", [128, len(m["triples"])], dt.float32,
                               kind="ExternalInput"))
    wblob_d = nc.inline_tensor(wblob_np, name="wblob")
    out_s = nc.dram_tensor("out_s", [NLOC["ph"], OUT_S], dt.float32,
                           kind="ExternalOutput")
    out_v = nc.dram_tensor("out_v", [NLOC["ph"], 3], dt.float32,
                           kind="ExternalOutput")

    tc_ctx = TileContext(nc)
    tc = tc_ctx.__enter__()
    import contextlib
    stack = contextlib.ExitStack()
    cpool = stack.enter_context(tc.tile_pool(name="cpool", bufs=1))
    wpool = stack.enter_context(tc.tile_pool(name="wpool", bufs=2))
    ppool = stack.enter_context(tc.tile_pool(name="ppool", bufs=1))
    work = stack.enter_context(tc.tile_pool(name="work", bufs=2))
    io = stack.enter_context(tc.tile_pool(name="io", bufs=1))
    pst = stack.enter_context(tc.tile_pool(name="pst", bufs=2, space="PSUM"))
    psm = stack.enter_context(tc.tile_pool(name="psm", bufs=1, space="PSUM"))
    psg = stack.enter_context(tc.tile_pool(name="psg", bufs=2, space="PSUM"))
    psa = stack.enter_context(tc.tile_pool(name="psa", bufs=2, space="PSUM"))
    dram = stack.enter_context(tc.tile_pool(name="dram", bufs=1, space="DRAM"))

    def wload(spec, name, pool=wpool, tag=None):
        off, K, M = spec
        t = pool.tile([128, M], dt.float32, name=name, tag=tag or name)
        nc.sync.dma_start(out=t[:], in_=wblob_d[:, off:off + M])
        return t, K, M

    ident, _, _ = wload(net["ident"], "ident", cpool)
    iota, _, _ = wload(net["iota"], "iota", cpool)
    sumh = {h: wload(net["sumh"][h], f"sum{h}", cpool)[0]
            for h in net["sumh"]}

    def transpose(dst_sb, src_ap, tag="tp"):
        # out = src.T via plain matmul against identity (the is_transpose
        # encoding only tolerates a single sync wait -> unusable under Tile).
        a, b = src_ap.shape
        pt = pst.tile([128, 128], dt.float32, name="tpt", tag=tag)
        nc.tensor.matmul(out=pt[:b, :a], lhsT=src_ap, rhs=ident[:a, :a],
                         start=True, stop=True)
        nc.vector.tensor_copy(out=dst_sb, in_=pt[:b, :a])

    # persistent feature-major node state
    sT = {nt: ppool.tile([128, NLOC[nt]], dt.float32, name=f"sT_{nt}")
          for nt in NLOC}
    vT = {nt: ppool.tile([VF, NLOC[nt]], dt.float32, name=f"vT_{nt}")
          for nt in NLOC}
    agg = {nt: ppool.tile([128, NDT[nt] * F], dt.float32, name=f"agg_{nt}")
           for nt in NLOC}

    for nt in ("ph", "pr"):
        for i in range(NDT[nt]):
            gt = work.tile([128, F], dt.float32, name="initg", tag="gath")
            nc.sync.dma_start(out=gt[:], in_=own[nt][i * 128:(i + 1) * 128, :])
            transpose(sT[nt][:, i * 128:(i + 1) * 128], gt[:, 0:S])
            transpose(vT[nt][:, i * 128:(i + 1) * 128], gt[:, S:F])

    gathered = [dram.tile([N_CORES * NLT, F], dt.float32, name=f"gathered{i}",
                          tag=f"gathered{i}", addr_space="Shared")
                for i in range(N_CONVS - 1)]
    contrib = [dram.tile([NLT, F], dt.float32, name=f"contrib{i}",
                         tag=f"contrib{i}") for i in range(N_CONVS - 1)]

    def load_gvp_weights(g, pfx, kind):
        wt = {}
        wt["Whe"] = [wload(s, f"{pfx}whe{i}", tag=f"Whe{i}_{kind}")
                     for i, s in enumerate(g["Whe"])]
        wt["Wue"] = wload(g["Wue"], f"{pfx}wue", tag=f"Wue_{kind}")
        wt["Ws"] = [wload(s, f"{pfx}ws{i}", tag=f"Ws{i}_{kind}")
                    for i, (s, k) in enumerate(g["Ws"])]
        wt["Wx"] = wload(g["Wx"], f"{pfx}wx", tag=f"Wx_{kind}")
        wt["Wg"] = wload(g["Wg"], f"{pfx}wg", tag=f"Wg_{kind}")
        wt["b"] = wload(g["b"], f"{pfx}b", tag=f"b_{kind}")
        wt["bg"] = wload(g["bg"], f"{pfx}bg", tag=f"bg_{kind}")
        return wt

    def gvp_chunk(g, wt, v_srcs, s_srcs, vec_sigmoid, pfx):
        """One 512-edge chunk of one GVP. v_srcs: [(ap, row0, nrows)] chunk APs
        (c-outer rows matching Whe). s_srcs: chunk APs matching g['Ws'] splits.
        Returns (s_out, v_out, gate) SBUF tiles [sout, w], [3*vout, w], [vout, w]."""
        h, vout, sout = g["h"], g["vout"], g["sout"]
        w = v_srcs[0].shape[-1]
        pvh = psm.tile([128, CH], dt.float32, name="pvh", tag="pvh")
        for i, ap in enumerate(v_srcs):
            whe, Kh, _ = wt["Whe"][i]
            nc.tensor.matmul(out=pvh[:3 * h, :w], lhsT=whe[:Kh, :3 * h],
                             rhs=ap, start=(i == 0), stop=(i == len(v_srcs) - 1))
        vh = work.tile([96, CH], dt.float32, name=f"{pfx}vh", tag="vh")
        nc.vector.tensor_copy(out=vh[:3 * h, :w], in_=pvh[:3 * h, :w])
        sq = work.tile([96, CH], dt.float32, name=f"{pfx}sq", tag="sq")
        nc.scalar.activation(out=sq[:3 * h, :w], in_=pvh[:3 * h, :w], func=AF.Square)
        pnrm = psg.tile([48, CH], dt.float32, name="pnrm", tag="gv")
        nc.tensor.matmul(out=pnrm[:h, :w], lhsT=sumh[h][:3 * h, :h],
                         rhs=sq[:3 * h, :w], start=True, stop=True)
        sh = work.tile([32, CH], dt.float32, name=f"{pfx}sh", tag="sh")
        nc.scalar.activation(out=sh[:h, :w], in_=pnrm[:h, :w], func=AF.Sqrt)
        ps = psm.tile([128, CH], dt.float32, name="ps", tag="ps")
        for i, ap in enumerate(s_srcs):
            wtile, K_, _ = wt["Ws"][i]
            nc.tensor.matmul(out=ps[:sout, :w], lhsT=wtile[:K_, :sout], rhs=ap,
                             start=(i == 0), stop=False)
        wx, Kx, _ = wt["Wx"]
        nc.tensor.matmul(out=ps[:sout, :w], lhsT=wx[:Kx, :sout], rhs=sh[:h, :w],
                         start=False, stop=True)
        s_out = work.tile([128, CH], dt.float32, name=f"{pfx}so", tag=f"so{pfx[-1]}")
        bt, _, _ = wt["b"]
        nc.scalar.activation(out=s_out[:sout, :w], in_=ps[:sout, :w], func=AF.Silu,
                             bias=bt[:sout, 0:1])
        wg, Kg, _ = wt["Wg"]
        pg = psg.tile([48, CH], dt.float32, name="pg", tag="gv")
        nc.tensor.matmul(out=pg[:3 * vout, :w], lhsT=wg[:Kg, :3 * vout],
                         rhs=s_out[:sout, :w], start=True, stop=True)
        gate = work.tile([48, CH], dt.float32, name=f"{pfx}gate", tag="gate")
        bgt, _, _ = wt["bg"]
        nc.scalar.activation(out=gate[:3 * vout, :w], in_=pg[:3 * vout, :w],
                             func=AF.Sigmoid if vec_sigmoid else AF.Identity,
                             bias=bgt[:3 * vout, 0:1])
        wue, Ku, _ = wt["Wue"]
        pvu = psg.tile([48, CH], dt.float32, name="pvu", tag="gv")
        nc.tensor.matmul(out=pvu[:3 * vout, :w], lhsT=wue[:Ku, :3 * vout],
                         rhs=vh[:3 * h, :w], start=True, stop=True)
        v_out = work.tile([48, CH], dt.float32, name=f"{pfx}vo", tag=f"vo{pfx[-1]}")
        nc.vector.tensor_mul(out=v_out[:3 * vout, :w], in0=pvu[:3 * vout, :w],
                             in1=gate[:3 * vout, :w])
        return s_out, v_out, gate

    # ---------------- conv layers
    for L in range(N_CONVS):
        lw = net["convs"][L]
        for nt in ("ph", "pr"):
            nc.gpsimd.memset(agg[nt][:], 0.0)
        for et, (st, dtp) in EDGES.items():
            m = meta[et]
            e_pad, ntiles, nchunks = m["e_pad"], m["ntiles"], m["nchunks"]
            gs = lw["msg"][et]
            wts = [load_gvp_weights(g, f"L{L}{et}g{i}", f"m{i}")
                   for i, g in enumerate(gs)]
            idx_sb = io.tile([128, ntiles], dt.int32, name=f"idx{et}", tag="idx")
            nc.sync.dma_start(out=idx_sb[:],
                              in_=ein[et]["idx"][0 if L == 0 else 1])
            ntri = len(m["triples"])
            rel_sb = io.tile([128, ntri], dt.float32, name=f"rel{et}", tag="rel")
            nc.sync.dma_start(out=rel_sb[:], in_=ein[et]["rel"][:])
            src_tab = tab[st][:] if L == 0 else gathered[L - 1][:]
            # group triples by block
            tri_by_blk = {}
            for k, (blk, dtile) in enumerate(m["triples"]):
                tri_by_blk.setdefault(blk, []).append((k, dtile))
            # first/last triple index per dtile
            first_tri, last_tri = {}, {}
            for k, (blk, dtile) in enumerate(m["triples"]):
                first_tri.setdefault(dtile, k)
                last_tri[dtile] = k
            pa_tiles = {}
            for ci in range(nchunks):
                c0 = ci * CH
                w = CH
                s0c = work.tile([128, CH], dt.float32, name="s0c", tag="s0c")
                vcc = work.tile([51, CH], dt.float32, name="vcc", tag="vcc")
                gsbs = []
                for j in range(4):
                    t = ci * 4 + j
                    gsb = work.tile([128, F], dt.float32, name="gsb", tag="gath")
                    nc.gpsimd.indirect_dma_start(
                        out=gsb[:], out_offset=None, in_=src_tab,
                        in_offset=bass.IndirectOffsetOnAxis(
                            ap=idx_sb[:, t:t + 1], axis=0))
                    gsbs.append(gsb)
                    transpose(s0c[:, j * 128:(j + 1) * 128], gsb[:, 0:S])
                    transpose(vcc[0:VF, j * 128:(j + 1) * 128], gsb[:, S:F])
                ch = slice(c0, c0 + w)
                # unit rows (vec index v=0) live at partitions 48:51; d at 0
                nc.sync.dma_start(out=vcc[VF:VF + 3, :w],
                                  in_=ein[et]["unitd"][1:4, ch])
                d_sb = work.tile([1, CH], dt.float32, name="d_sb", tag="d_sb")
                nc.sync.dma_start(out=d_sb[:, :w], in_=ein[et]["unitd"][0:1, ch])
                s1, v1, _ = gvp_chunk(
                    gs[0], wts[0], v_srcs=[vcc[:, :w]],
                    s_srcs=[s0c[:, :w], d_sb[:, :w]],
                    vec_sigmoid=True, pfx="m0")
                s2, v2, _ = gvp_chunk(
                    gs[1], wts[1], v_srcs=[v1[:VF, :w]],
                    s_srcs=[s1[:S, :w]], vec_sigmoid=True, pfx="m1")
                s3, v3, _ = gvp_chunk(
                    gs[2], wts[2], v_srcs=[v2[:VF, :w]],
                    s_srcs=[s2[:S, :w]], vec_sigmoid=True, pfx="m2")
                # scatter this chunk's 4 blocks
                for j in range(4):
                    blk = ci * 4 + j
                    em = work.tile([128, F], dt.float32, name="em", tag="em", bufs=3)
                    transpose(em[:, 0:S], s3[:S, j * 128:(j + 1) * 128])
                    transpose(em[:, S:F], v3[:VF, j * 128:(j + 1) * 128])
                    for (k, dtile) in tri_by_blk.get(blk, []):
                        ind = work.tile([128, 128], dt.float32, name="ind",
                                        tag="ind", bufs=2)
                        nc.vector.tensor_tensor(
                            out=ind[:],
                            in0=rel_sb[:, k:k + 1].to_broadcast([128, 128]),
                            in1=iota[:, 0:128], op=OP.is_equal)
                        if k == first_tri[dtile]:
                            pa_tiles[dtile] = psa.tile([128, F], dt.float32,
                                                       name="pa", tag="pa")
                        nc.tensor.matmul(out=pa_tiles[dtile][:], lhsT=ind[:],
                                         rhs=em[:], start=(k == first_tri[dtile]),
                                         stop=(k == last_tri[dtile]))
                        if k == last_tri[dtile]:
                            a = agg[dtp][:, dtile * F:(dtile + 1) * F]
                            nc.vector.scalar_tensor_tensor(
                                out=a, in0=pa_tiles[dtile][:],
                                scalar=1.0 / MSG_NORM, in1=a,
                                op0=OP.mult, op1=OP.add)
                            del pa_tiles[dtile]
        # ---- node updates
        for nt in ("ph", "pr"):
            gs = lw["upd"][nt]
            wts = [load_gvp_weights(g, f"L{L}u{nt}g{i}", f"u{i}")
                   for i, g in enumerate(gs)]
            nloc = NLOC[nt]
            for ci in range(max(nloc // CH, 1)):
                w = min(CH, nloc)
                c0 = ci * CH
                aggTs = work.tile([128, CH], dt.float32, name="aggTs", tag="aggTs")
                aggTv = work.tile([VF, CH], dt.float32, name="aggTv", tag="aggTv")
                for j in range(w // 128):
                    dtile = (c0 + j * 128) // 128
                    a = agg[nt][:, dtile * F:(dtile + 1) * F]
                    transpose(aggTs[:, j * 128:(j + 1) * 128], a[:, 0:S])
                    transpose(aggTv[:, j * 128:(j + 1) * 128], a[:, S:F])
                ch = slice(c0, c0 + w)
                s1, v1, _ = gvp_chunk(
                    gs[0], wts[0],
                    v_srcs=[vT[nt][:, ch], aggTv[:, :w]],
                    s_srcs=[sT[nt][:, ch], aggTs[:, :w]],
                    vec_sigmoid=True, pfx="u0")
                s2, v2, _ = gvp_chunk(
                    gs[1], wts[1], v_srcs=[v1[:VF, :w]],
                    s_srcs=[s1[:S, :w]], vec_sigmoid=True, pfx="u1")
                nc.vector.tensor_add(out=sT[nt][:, ch], in0=sT[nt][:, ch],
                                     in1=s2[:S, :w])
                nc.vector.tensor_add(out=vT[nt][:, ch], in0=vT[nt][:, ch],
                                     in1=v2[:VF, :w])
        # ---- exchange
        if L < N_CONVS - 1:
            for nt, row0 in (("ph", 0), ("pr", NLOC["ph"])):
                for i in range(NDT[nt]):
                    cs = work.tile([128, F], dt.float32, name="cs", tag="gath")
                    transpose(cs[:, 0:S], sT[nt][:, i * 128:(i + 1) * 128])
                    transpose(cs[:, S:F], vT[nt][:, i * 128:(i + 1) * 128])
                    r = row0 + i * 128
                    nc.sync.dma_start(out=contrib[L][r:r + 128, :], in_=cs[:])
            nc.gpsimd.collective_compute(
                "AllGather", mybir.AluOpType.bypass,
                replica_groups=[list(range(N_CORES))],
                ins=[contrib[L][:].opt()], outs=[gathered[L][:].opt()])

    # ---------------- noise head (pharm local, 512 nodes = 1 chunk)
    w = NLOC["ph"]
    gs = net["noise"]
    wts = [load_gvp_weights(g, f"ng{i}", f"n{i}") for i, g in enumerate(gs)]
    s1, v1, _ = gvp_chunk(gs[0], wts[0], v_srcs=[vT["ph"][:, :w]],
                          s_srcs=[sT["ph"][:, :w]], vec_sigmoid=True, pfx="n0")
    s2, v2, _ = gvp_chunk(gs[1], wts[1], v_srcs=[v1[:VF, :w]],
                          s_srcs=[s1[:S, :w]], vec_sigmoid=True, pfx="n1")
    s3, v3, _ = gvp_chunk(gs[2], wts[2], v_srcs=[v2[:VF, :w]],
                          s_srcs=[s2[:S, :w]], vec_sigmoid=False, pfx="n2")
    wout, Ko, _ = wload(net["W_out"], "wout")
    brep, _, _ = wload(net["b_out_rep"], "brep")
    for i in range(NDT["ph"]):
        pm = psm.tile([128, CH], dt.float32, name="pm", tag="ps")
        nc.tensor.matmul(out=pm[:, :OUT_S],
                         lhsT=s3[:INTER_S, i * 128:(i + 1) * 128],
                         rhs=wout[:Ko, :OUT_S], start=True, stop=True)
        osb = work.tile([128, OUT_S], dt.float32, name="osb", tag="gath")
        nc.vector.tensor_add(out=osb[:], in0=pm[:, :OUT_S], in1=brep[:, :OUT_S])
        nc.sync.dma_start(out=out_s[i * 128:(i + 1) * 128, :], in_=osb[:])
        ovb = work.tile([128, 3], dt.float32, name="ovb", tag="ovb")
        transpose(ovb[:], v3[0:3, i * 128:(i + 1) * 128])
        nc.sync.dma_start(out=out_v[i * 128:(i + 1) * 128, :], in_=ovb[:])

    stack.close()
    tc_ctx.__exit__(None, None, None)
    nc.compile()
    return nc


# ------------------------------------------------------------------ entrypoint
def _run(inputs, trace=False):
    params = inputs["params"]
    net, wblob = _prep_params(params)
    tab_ph, tab_pr = _pack_tables(inputs["pharm_s"], inputs["pharm_v"],
                                  inputs["prot_s"], inputs["prot_v"])
    percore, meta = _prep_edges(inputs)
    nc = build_program(net, wblob, meta)

    in_maps = []
    for c in range(N_CORES):
        m = dict(percore[c])
        m["tab_ph"] = tab_ph
        m["tab_pr"] = tab_pr
        m["own_ph"] = np.ascontiguousarray(
            tab_ph[c * NLOC["ph"]:(c + 1) * NLOC["ph"]])
        m["own_pr"] = np.ascontiguousarray(
            tab_pr[c * NLOC["pr"]:(c + 1) * NLOC["pr"]])
        in_maps.append(m)

    from concourse.bass_utils import run_bass_kernel_spmd
    res = run_bass_kernel_spmd(nc, in_maps, core_ids=list(range(N_CORES)),
                               trace=trace)
    scalar = np.concatenate([res.results[c]["out_s"] for c in range(N_CORES)], axis=0)
    vector = np.concatenate([res.results[c]["out_v"] for c in range(N_CORES)], axis=0)
    return scalar, vector, res


def kernel(**inputs):
    scalar, vector, _ = _run(inputs, trace=False)
    return scalar, vector


def kernel_traced(**inputs):
    scalar, vector, res = _run(inputs, trace=True)
    return scalar, vector, res.exec_time_ns


def kernel_bench(reps=5, **inputs):
    """Correctness outputs + warm per-call device time (jit cached, inputs
    device-resident; includes the axon RPC floor)."""
    import time

    import jax
    from jax.sharding import Mesh, PartitionSpec
    from jax.experimental.shard_map import shard_map

    from concourse import bass2jax, mybir

    params = inputs["params"]
    net, wblob = _prep_params(params)
    tab_ph, tab_pr = _pack_tables(inputs["pharm_s"], inputs["pharm_v"],
                                  inputs["prot_s"], inputs["prot_v"])
    percore, meta = _prep_edges(inputs)
    nc = build_program(net, wblob, meta)
    in_maps = []
    for c in range(N_CORES):
        m = dict(percore[c])
        m["tab_ph"] = tab_ph
        m["tab_pr"] = tab_pr
        m["own_ph"] = np.ascontiguousarray(tab_ph[c * NLOC["ph"]:(c + 1) * NLOC["ph"]])
        m["own_pr"] = np.ascontiguousarray(tab_pr[c * NLOC["pr"]:(c + 1) * NLOC["pr"]])
        in_maps.append(m)

    bass2jax.install_neuronx_cc_hook()
    partition_name = nc.partition_id_tensor.name if nc.partition_id_tensor else None
    in_names, out_names, out_avals, zero_outs = [], [], [], []
    for alloc in nc.m.functions[0].allocations:
        if not isinstance(alloc, mybir.MemoryLocationSet):
            continue
        name = alloc.memorylocations[0].name
        if alloc.kind == "ExternalInput":
            if name != partition_name:
                in_names.append(name)
        elif alloc.kind == "ExternalOutput":
            out_names.append(name)
            shape = tuple(alloc.tensor_shape)
            dtype = mybir.dt.np(alloc.dtype)
            out_avals.append(jax.core.ShapedArray(shape, dtype))
            zero_outs.append(np.zeros(shape, dtype))
    n_params = len(in_names)
    all_names = in_names + out_names + ([partition_name] if partition_name else [])

    def _body(*args):
        operands = list(args)
        if partition_name is not None:
            operands.append(bass2jax.partition_id_tensor())
        return tuple(bass2jax._bass_exec_p.bind(
            *operands, out_avals=tuple(out_avals), in_names=tuple(all_names),
            out_names=tuple(out_names), lowering_input_output_aliases=(),
            sim_require_finite=True, sim_require_nnan=True, nc=nc))

    devices = jax.devices()[:N_CORES]
    mesh = Mesh(np.asarray(devices), ("core",))
    nspec = n_params + len(out_names)
    fn = jax.jit(shard_map(_body, mesh=mesh,
                           in_specs=(PartitionSpec("core"),) * nspec,
                           out_specs=(PartitionSpec("core"),) * len(out_names),
                           check_rep=False), keep_unused=True)
    concat_in = [np.concatenate([np.asarray(in_maps[c][k]) for c in range(N_CORES)],
                                axis=0) for k in in_names]
    concat_zeros = [np.zeros((N_CORES * z.shape[0], *z.shape[1:]), z.dtype)
                    for z in zero_outs]
    dev_in = [jax.device_put(x) for x in concat_in + concat_zeros]
    outs = fn(*dev_in)
    jax.block_until_ready(outs)
    times = []
    for _ in range(reps):
        t0 = time.perf_counter()
        outs = fn(*dev_in)
        jax.block_until_ready(outs)
        times.append(time.perf_counter() - t0)
    res = {name: np.asarray(outs[i]).reshape(N_CORES, *out_avals[i].shape)
           for i, name in enumerate(out_names)}
    scalar = res["out_s"].reshape(N_PH, OUT_S)
    vector = res["out_v"].reshape(N_PH, 3)
    ns = int(min(times) * 1e9)
    print("bench times (ms):", [f"{t*1e3:.2f}" for t in times])
    return scalar, vector, ns
